# revision 2
# baseline (speedup 1.0000x reference)
"""Trainium2 Bass kernel for nn_AggregPolicy_85169201480382 (final).

Self-contained: takes FULL inputs (B=2097152), shards the batch over 8
NeuronCores (pure data parallelism), runs a Bass/Tile kernel per core, and
gathers the full [B,7,1] output.

v8: stride-2 diagonal schedule + cross-wave fused pointwise.

Schedule: at step t, run iteration `it` of supergroup s = t - 2*it (slot
sigma = 6-it).  Phase1 per wave: 2 matmuls (rz->pa, ih->pb), sigmoid
(pa->rz-slab slot), t8 (r*hn), x8 (t+i_n).  Phase2 fused across all valid
slots of the step: ONE tanh, ONE d, ONE e, ONE h' (plus 2 small it0-d ops).
h'(t) is consumed at t+2, so phase2(t) overlaps phase1(t+1) — no barrier.

Gate layout per slab (c-major): iter rhs [rz: r96|z96 | ih: in96|hn96],
col = 96*c + 32*i + su.  init rhs (N=480): [rz 192 | h0_b01 64 | ih 192 |
h0_b2 32] -> MM-a N=256 into pa, MM-b N=224 into pb.
PSUM: one pool, 4 tiles x [128,1024] (2 banks); slabs at 256-stride.
"""
import os
import sys

for p in ("/opt/trn_rl_repo", "/root/.axon_site/_ro/trn_rl_repo"):
    if p not in sys.path:
        sys.path.insert(0, p)

import numpy as np
import ml_dtypes

import concourse.bass as bass
import concourse.bacc as bacc
import concourse.mybir as mybir
from concourse.tile import TileContext
from concourse.bass_utils import run_bass_kernel_spmd

FP32 = mybir.dt.float32
DT16 = mybir.dt.float16
AF = mybir.ActivationFunctionType

N_CORES = 8
G = 4
NIT = 7

LA_ENV = int(os.environ.get("KV8_LA", "4"))
NHP_ENV = int(os.environ.get("KV8_NHP", "5"))
HF_ENV = int(os.environ.get("KV8_HF", "6"))
HFS_ENV = int(os.environ.get("KV8_HFS", "4"))
XN_ENV = int(os.environ.get("KV8_XN", "8"))
PWB_ENV = int(os.environ.get("KV8_PWB", "2"))
OUTS_ENV = int(os.environ.get("KV8_OUTS", "6"))
E_ENGINE = os.environ.get("KV8_E", "dve")    # dve | pool
D_ENGINE = os.environ.get("KV8_D", "dve")    # dve | pool
EXTN = int(os.environ.get("KV8_EXTN", "2"))  # waves/step with ACT i_n-copy


def _su(u, k):
    return 4 * u + k


def _build_w_all(w):
    f8 = lambda a: np.asarray(a, np.float64)
    Wih_j, Whh_j = f8(w["Wih_j"]), f8(w["Whh_j"])
    bih_j, bhh_j = f8(w["bih_j"]), f8(w["bhh_j"])
    Wih_m, Whh_m = f8(w["Wih_m"]), f8(w["Whh_m"])
    bih_m, bhh_m = f8(w["bih_m"]), f8(w["bhh_m"])

    W_all = np.zeros((33, 128), np.float64)  # row 32 = bias; col = 32c+su

    def col(c, u, k):
        return 32 * c + 4 * u + k

    for k in range(4):
        for f in range(4):
            W_all[_su(1, f), col(0, 0, k)] += Wih_m[k, f]
            W_all[_su(0, f), col(0, 0, k)] += Whh_m[k, f]
            W_all[_su(1, f), col(1, 0, k)] += Wih_m[4 + k, f]
            W_all[_su(0, f), col(1, 0, k)] += Whh_m[4 + k, f]
            W_all[_su(1, f), col(2, 0, k)] += Wih_m[8 + k, f]
            W_all[_su(0, f), col(3, 0, k)] += Whh_m[8 + k, f]
        W_all[32, col(0, 0, k)] += bih_m[k] + bhh_m[k]
        W_all[32, col(1, 0, k)] += bih_m[4 + k] + bhh_m[4 + k]
        W_all[32, col(2, 0, k)] += bih_m[8 + k]
        W_all[32, col(3, 0, k)] += bhh_m[8 + k]

    for j in range(7):
        u = 1 + j
        for k in range(4):
            for f in range(4):
                W_all[_su(j, f), col(0, u, k)] += Wih_j[k, f]
                W_all[_su(j, f), col(1, u, k)] += Wih_j[4 + k, f]
                W_all[_su(j, f), col(2, u, k)] += Wih_j[8 + k, f]
                if j < 6:
                    W_all[_su(j + 2, f), col(0, u, k)] += Wih_j[k, 4 + f]
                    W_all[_su(j + 2, f), col(1, u, k)] += Wih_j[4 + k, 4 + f]
                    W_all[_su(j + 2, f), col(2, u, k)] += Wih_j[8 + k, 4 + f]
                W_all[_su(u, f), col(0, u, k)] += Whh_j[k, f]
                W_all[_su(u, f), col(1, u, k)] += Whh_j[4 + k, f]
                W_all[_su(u, f), col(3, u, k)] += Whh_j[8 + k, f]
            W_all[32, col(0, u, k)] += bih_j[k] + bhh_j[k]
            W_all[32, col(1, u, k)] += bih_j[4 + k] + bhh_j[4 + k]
            W_all[32, col(2, u, k)] += bih_j[8 + k]
            W_all[32, col(3, u, k)] += bhh_j[8 + k]
    return W_all


def _build_constants(w):
    f8 = lambda a: np.asarray(a, np.float64)
    Wj_init, bj_init = f8(w["Wj_init"]), f8(w["bj_init"])
    Wm_init, bm_init = f8(w["Wm_init"]), f8(w["bm_init"])
    Wact, bact = f8(w["Wact"]), f8(w["bact"])

    W_all = _build_w_all(w)

    Winit = np.zeros((20, 32), np.float64)  # row 19 = bias
    for k in range(4):
        for f in range(5):
            Winit[f, _su(0, k)] = Wm_init[k, f]
        Winit[19, _su(0, k)] = bm_init[k]
        for j in range(7):
            Winit[5 + j, _su(1 + j, k)] = Wj_init[k, 0]
            Winit[12 + j, _su(1 + j, k)] = Wj_init[k, 1]
            Winit[19, _su(1 + j, k)] = bj_init[k]

    # c-major remap: old col 32c+su -> new col 96c' + 32i + su layout is done
    # at assembly: iter rhs [128, 384]: rz region 96c+32i+su (c<2),
    # ih region 192 + 96(c-2)+32i+su.
    def newcol_iter(i, c, su):
        if c < 2:
            return 96 * c + 32 * i + su
        return 192 + 96 * (c - 2) + 32 * i + su

    rhs_iter = np.zeros((128, 384), np.float64)
    for i in range(3):
        for c in range(4):
            for su in range(32):
                nc_ = newcol_iter(i, c, su)
                rhs_iter[32 * i:32 * i + 32, nc_] = W_all[:32, 32 * c + su]
                rhs_iter[96, nc_] = W_all[32, 32 * c + su]

    # init rhs [128, 480]: [rz 0:192 | h0_b01 192:256 | ih 256:448 | h0_b2 448:480]
    C1 = Winit[:19] @ W_all[:32]
    b1 = Winit[19] @ W_all[:32] + W_all[32]

    def newcol_init(i, c, su):
        if c < 2:
            return 96 * c + 32 * i + su
        return 256 + 96 * (c - 2) + 32 * i + su

    rhs_init = np.zeros((128, 480), np.float64)
    for i in range(3):
        for c in range(4):
            for su in range(32):
                nc_ = newcol_init(i, c, su)
                rhs_init[32 * i:32 * i + 19, nc_] = C1[:, 32 * c + su]
                rhs_init[96, nc_] = b1[32 * c + su]
        base = 192 + 32 * i if i < 2 else 448
        for k_ in range(32):
            rhs_init[32 * i:32 * i + 19, base + k_] = Winit[:19, k_]
            rhs_init[96, base + k_] = Winit[19, k_]

    rhs_fin = np.zeros((128, 24), np.float64)
    for i in range(3):
        for j in range(7):
            for k in range(4):
                rhs_fin[32 * i + _su(1 + j, k), 8 * i + j] = Wact[0, k]
            rhs_fin[96, 8 * i + j] = bact[0]

    return rhs_iter, rhs_init, rhs_fin


def _shard_geometry(S):
    nblk = S // 128
    ngrp = (nblk + 2) // 3
    nsg = (ngrp + G - 1) // G
    return nblk, ngrp, nsg


def _pack_x(x_shard):
    S = x_shard.shape[0]
    nblk, ngrp, nsg = _shard_geometry(S)
    nslab = nsg * G
    v = np.zeros((nslab * 3, 128, 19), np.float32)
    v[:nblk] = x_shard.reshape(nblk, 128, 19)
    out = np.zeros((nslab, 128, 4, 32), dtype=np.float16)
    out[:, :, 0:3, 0:19] = v.reshape(nslab, 3, 128, 19).transpose(0, 2, 1, 3)
    out = out.reshape(nslab, 128, 128)
    out[:, :, 96] = 1.0
    out = out.reshape(nsg, G, 128, 128).transpose(0, 2, 1, 3)
    return np.ascontiguousarray(out.reshape(nsg * 128, G * 128))


# ---------------------------------------------------------------------------
def _build_nc(S, n_iters=NIT):
    assert S % 128 == 0
    nblk, ngrp, nsg = _shard_geometry(S)

    nc = bacc.Bacc(debug=False)
    xp = nc.dram_tensor("xp", [nsg * 128, G * 128], DT16, kind="ExternalInput")
    r_it = nc.dram_tensor("rhs_iter", [128, 384], DT16, kind="ExternalInput")
    r_in = nc.dram_tensor("rhs_init", [128, 480], DT16, kind="ExternalInput")
    r_fi = nc.dram_tensor("rhs_fin", [128, 24], DT16, kind="ExternalInput")
    acts = nc.dram_tensor("acts", [128, nsg * G * 3 * 8], FP32,
                          kind="ExternalOutput")

    acts_v = acts.ap().rearrange("m (b j) -> m b j", j=8)

    NS = n_iters            # slots per step
    SLOT = 384              # pointwise slot width (4 slabs x 96 state cols)
    HSLOT = 512             # hp slot width (4 slabs x 128)

    with TileContext(nc) as tc:
        with (
            tc.tile_pool(name="const", bufs=1) as cpool,
            tc.tile_pool(name="xn", bufs=XN_ENV) as xnpool,
            tc.tile_pool(name="hp", bufs=NHP_ENV) as hppool,
            tc.tile_pool(name="hf", bufs=HF_ENV) as hfpool,
            tc.tile_pool(name="hfs", bufs=HFS_ENV) as hfspool,
            tc.tile_pool(name="pw", bufs=PWB_ENV) as pwpool,
            tc.tile_pool(name="outs", bufs=OUTS_ENV) as opool,
            tc.tile_pool(name="psum", bufs=4, space="PSUM") as pspool,
        ):
            Wit = cpool.tile([128, 384], DT16, tag="wit")
            Win = cpool.tile([128, 480], DT16, tag="win")
            Wfi = cpool.tile([128, 24], DT16, tag="wfi")
            nc.sync.dma_start(out=Wit[:], in_=r_it.ap())
            nc.sync.dma_start(out=Win[:], in_=r_in.ap())
            nc.sync.dma_start(out=Wfi[:], in_=r_fi.ap())

            # hp slabs: [128, NS*512]; per 128-block col 96 = 1.0 (bias row)
            HP = [hppool.tile([128, NS * HSLOT], DT16, tag="hp",
                              name=f"hpr{i}") for i in range(NHP_ENV)]
            for tl in HP:
                nc.vector.memset(tl[:], 0.0)
                for g in range(NS * G):
                    nc.vector.memset(tl[:, 128 * g + 96:128 * g + 97], 1.0)

            xp_m = xp.ap().rearrange("(s m) c -> m s c", m=128)
            xn_t = {}
            hf_t = {}           # sg -> current feature-major tile
            hp_of = {}          # sg -> (hp_tile, slot) of its latest h'

            def load_sg(s):
                xn = xnpool.tile([128, G * 128], DT16, tag="xn")
                nc.scalar.dma_start(
                    out=xn[:].rearrange("p (s c) -> p s c", c=G * 128)[:, 0:1],
                    in_=xp_m[:, s:s + 1])
                xn_t[s] = xn

            def xf_sg(s):
                xf = hfpool.tile([128, G * 128], DT16, tag="hfx")
                nc.sync.dma_start_transpose(
                    out=xf[:].rearrange("p (g c) -> p g c", c=128),
                    in_=xn_t.pop(s)[:])
                hf_t[s] = xf[:]

            def fin(s):
                hf_cur = hf_t.pop(s)
                hp_of.pop(s, None)
                psf = pspool.tile([128, 1024], FP32, tag="ps")
                for j in range(G):
                    nc.tensor.matmul(
                        out=psf[:, 24 * j:24 * (j + 1)],
                        lhsT=hf_cur[:, 128 * j:128 * (j + 1)],
                        rhs=Wfi[:, 0:24], start=True, stop=True)
                ao = opool.tile([128, G * 24], FP32, tag="ao")
                nc.scalar.activation(ao[:, 0:96], psf[:, 0:96], AF.Copy)
                nc.scalar.dma_start(
                    out=acts_v[:, s * 3 * G:(s + 1) * 3 * G, :],
                    in_=ao[:, 0:96].rearrange("p (b c) -> p b c", c=8))

            def phase1(t):
                if t + LA_ENV < nsg:
                    load_sg(t + LA_ENV)
                if t + 1 < nsg and (t + 1) not in hf_t:
                    xf_sg(t + 1)

                # valid iterations this step: s = t - 2*it in [0, nsg)
                its = [it for it in range(n_iters)
                       if 0 <= t - 2 * it < nsg]
                if not its:
                    return None
                sl = {it: n_iters - 1 - it for it in its}   # slot index

                rz = pwpool.tile([128, NS * 2 * SLOT], DT16, tag="rz")
                ihs = (pwpool.tile([128, NS * SLOT], DT16, tag="ihs",
                                   name="ihs") if EXTN else None)
                tt = pwpool.tile([128, NS * SLOT], DT16, tag="tt")
                xx = pwpool.tile([128, NS * SLOT], DT16, tag="xx")
                nn = pwpool.tile([128, NS * SLOT], DT16, tag="nn")
                dd = pwpool.tile([128, NS * SLOT], DT16, tag="dd")
                ee = pwpool.tile([128, NS * SLOT], DT16, tag="ee")
                hpt = HP[t % NHP_ENV]

                pb_of = {}
                pa_of = {}
                # ---- phase 1 (per wave): MMs, sigmoid, t8, x8 ----
                for it in its:
                    s = t - 2 * it
                    sg = sl[it]
                    hf_cur = hf_t[s]
                    pa = pspool.tile([128, 1024], FP32, tag="ps")
                    pb = pspool.tile([128, 1024], FP32, tag="ps")
                    pa_of[it], pb_of[it] = pa, pb
                    W, na, nb, ihoff = ((Win, 256, 224, 256) if it == 0
                                        else (Wit, 192, 192, 192))
                    for g in range(4):
                        nc.tensor.matmul(
                            out=pa[:, 256 * g:256 * g + na],
                            lhsT=hf_cur[:, 128 * g:128 * (g + 1)],
                            rhs=W[:, 0:na], start=True, stop=True)
                    for g in range(4):
                        nc.tensor.matmul(
                            out=pb[:, 256 * g:256 * g + nb],
                            lhsT=hf_cur[:, 128 * g:128 * (g + 1)],
                            rhs=W[:, ihoff:ihoff + nb], start=True, stop=True)

                    pav = pa[:].rearrange("p (g c) -> p g c", c=256)
                    pbv = pb[:].rearrange("p (g c) -> p g c", c=256)
                    rzs = rz[:, 2 * SLOT * sg:2 * SLOT * (sg + 1)].rearrange(
                        "p (g c) -> p g c", c=192)
                    tts = tt[:, SLOT * sg:SLOT * (sg + 1)].rearrange(
                        "p (g c) -> p g c", c=96)
                    xxs = xx[:, SLOT * sg:SLOT * (sg + 1)].rearrange(
                        "p (g c) -> p g c", c=96)
                    use_ext = EXTN and it >= n_iters - EXTN and it != 0
                    if use_ext:
                        iss = ihs[:, SLOT * sg:SLOT * (sg + 1)].rearrange(
                            "p (g c) -> p g c", c=96)
                        nc.scalar.activation(iss, pbv[:, :, 0:96], AF.Copy)
                    nc.scalar.activation(rzs, pav[:, :, 0:192], AF.Sigmoid)
                    nc.vector.tensor_mul(out=tts, in0=rzs[:, :, 0:96],
                                         in1=pbv[:, :, 96:192])
                    nc.vector.tensor_add(out=xxs, in0=tts,
                                         in1=iss if use_ext
                                         else pbv[:, :, 0:96])
                    if it == 0:
                        # it0's d reads psum h0 regions: do tanh+d inline so
                        # the psum tiles free within phase 1 (avoids a
                        # psum-pool allocation cycle against fused phase 2)
                        nns = nn[:, SLOT * sg:SLOT * (sg + 1)].rearrange(
                            "p (g c) -> p g c", c=96)
                        dds = dd[:, SLOT * sg:SLOT * (sg + 1)].rearrange(
                            "p (g c) -> p g c", c=96)
                        nc.scalar.activation(nns, xxs, AF.Tanh)
                        nc.vector.tensor_sub(out=dds[:, :, 0:64],
                                             in0=pav[:, :, 192:256],
                                             in1=nns[:, :, 0:64])
                        nc.vector.tensor_sub(out=dds[:, :, 64:96],
                                             in0=pbv[:, :, 192:224],
                                             in1=nns[:, :, 64:96])

                return dict(t=t, its=its, sl=sl, rz=rz, tt=tt, xx=xx,
                            nn=nn, dd=dd, ee=ee, hpt=hpt)

            def phase2(ctx):
                t, its, sl = ctx["t"], ctx["its"], ctx["sl"]
                rz, xx, nn = ctx["rz"], ctx["xx"], ctx["nn"]
                dd, ee, hpt = ctx["dd"], ctx["ee"], ctx["hpt"]
                s0, s1 = min(sl.values()), max(sl.values())
                t1 = s1 - 1 if 0 in its else s1     # it0 slot already tanh'd
                its_g1 = [it for it in its if it >= 1]
                hp_prev = HP[(t - 2) % NHP_ENV]
                hpv = hp_prev[:].rearrange("p (q g c) -> p q g c", g=G, c=128)
                nv = nn[:].rearrange("p (q g c) -> p q g c", g=G, c=96)
                dv = dd[:].rearrange("p (q g c) -> p q g c", g=G, c=96)
                d_eng = nc.gpsimd if D_ENGINE == "pool" else nc.vector
                # chunked tanh/d so d(lo) overlaps tanh(hi) on ACT
                if t1 >= s0:
                    mid = (s0 + t1) // 2
                    chunks = ([(s0, mid), (mid + 1, t1)] if t1 > s0
                              else [(s0, t1)])
                    for (a, b) in chunks:
                        nc.scalar.activation(nn[:, SLOT * a:SLOT * (b + 1)],
                                             xx[:, SLOT * a:SLOT * (b + 1)],
                                             AF.Tanh)
                        if its_g1:
                            ga = max(a, min(sl[i] for i in its_g1))
                            gb = min(b, max(sl[i] for i in its_g1))
                            if ga <= gb:
                                d_eng.tensor_sub(
                                    out=dv[:, ga:gb + 1],
                                    in0=hpv[:, ga + 1:gb + 2, :, 0:96],
                                    in1=nv[:, ga:gb + 1])
                rzv = rz[:].rearrange("p (q g c) -> p q g c", g=G, c=192)
                dv = dd[:].rearrange("p (q g c) -> p q g c", g=G, c=96)
                ev = ee[:].rearrange("p (q g c) -> p q g c", g=G, c=96)
                e_eng = nc.gpsimd if E_ENGINE == "pool" else nc.vector
                e_eng.tensor_mul(out=ev[:, s0:s1 + 1],
                                 in0=rzv[:, s0:s1 + 1, :, 96:192],
                                 in1=dv[:, s0:s1 + 1])

                hpv_t = hpt[:].rearrange("p (q g c) -> p q g c", g=G, c=128)
                nv = nn[:].rearrange("p (q g c) -> p q g c", g=G, c=96)
                nc.vector.tensor_add(out=hpv_t[:, s0:s1 + 1, :, 0:96],
                                     in0=nv[:, s0:s1 + 1],
                                     in1=ev[:, s0:s1 + 1])

                # ---- ONE batched transpose: h' slots -> feature-major ----
                hfs = hfspool.tile([128, NS * HSLOT], DT16, tag="hfs")
                nc.sync.dma_start_transpose(
                    out=hfs[:, HSLOT * s0:HSLOT * (s1 + 1)].rearrange(
                        "p (g c) -> p g c", c=128),
                    in_=hpt[:, HSLOT * s0:HSLOT * (s1 + 1)])
                for it in its:
                    s = t - 2 * it
                    sg = sl[it]
                    hf_t[s] = hfs[:, HSLOT * sg:HSLOT * (sg + 1)]
                    hp_of[s] = (hpt, sg)

                if t - 2 * n_iters >= 0:
                    fin(t - 2 * n_iters)

            # prologue
            for s in range(min(LA_ENV, nsg)):
                load_sg(s)
            if nsg > 0:
                xf_sg(0)
            for t in range(nsg + 2 * n_iters):
                ctx = phase1(t)
                if ctx is not None:
                    phase2(ctx)
                elif t - 2 * n_iters >= 0:
                    fin(t - 2 * n_iters)

    nc.compile()
    return nc


_NC_CACHE = {}


def _get_nc(S):
    if S not in _NC_CACHE:
        _NC_CACHE[S] = _build_nc(S)
    return _NC_CACHE[S]


def prepare_run(inputs):
    x = np.ascontiguousarray(np.asarray(inputs["x"], np.float32))
    B = x.shape[0]
    assert B % N_CORES == 0
    S = B // N_CORES

    rhs_iter, rhs_init, rhs_fin = _build_constants(inputs)
    bf = lambda a: np.ascontiguousarray(np.asarray(a, np.float16))
    consts = {
        "rhs_iter": bf(rhs_iter),
        "rhs_init": bf(rhs_init),
        "rhs_fin": bf(rhs_fin),
    }

    nc = _get_nc(S)
    in_maps = [
        {"xp": _pack_x(x[c * S:(c + 1) * S]), **consts} for c in range(N_CORES)
    ]
    return nc, in_maps


def _unpack_acts(a, S):
    nblk = S // 128
    a = a.reshape(128, -1, 8)[:, :nblk, :7]
    return a.transpose(1, 0, 2).reshape(S, 7)


def kernel(**inputs):
    B = np.asarray(inputs["x"]).shape[0]
    S = B // N_CORES
    nc, in_maps = prepare_run(inputs)
    res = run_bass_kernel_spmd(nc, in_maps, core_ids=list(range(N_CORES)))
    out = np.concatenate(
        [_unpack_acts(res.results[c]["acts"], S) for c in range(N_CORES)], axis=0)
    return out.reshape(B, 7, 1).astype(np.float32)
